# revision 4
# baseline (speedup 1.0000x reference)
"""CrossEntropyLossWithProb on 8 trn2 NeuronCores.

loss = -mean(log(max(probs[i, labels[i]], 1e-8)))  over i in [0, 8192)

Sharding: data-parallel over the batch (row) axis, 1024 rows per core.
Only the 1024 addressed probabilities per core are ever read from HBM
(4 KB out of the 128 MB shard) via indirect (gathering) DMA - the kernel
is latency-bound, not bandwidth-bound.

Per-core device kernel (raw bacc, manual semaphores, one exit barrier):
  SP : dma idx tile [128,8] -> s_idx; wait s_act; dma out [128,1] -> s_out;
       wait s_out (output landed)
  PL : wait s_idx; 8x indirect gather (HW consumes one index per SBUF
       partition per instruction, so each call gathers 128 elements)
  DVE: memset ln-bias 0; wait gathers; clamp to 1e-8
  ACT: (act-table load off critical path) ln with free-axis accumulate
  tail: all-engine barrier, then DMA-state reset + semaphore clear so the
       NEFF can be re-executed.

Host: flat element indices row*V+label are computed per shard when
building inputs (index preprocessing); the 8 x [128,1] partial sums are
summed on host for the final mean (replaces the all-reduce).
"""

import numpy as np

import concourse.bacc as bacc
import concourse.bass as bass
import concourse.mybir as mybir
from concourse.bass import compact_to_ranges

B, V = 8192, 32000
N_CORES = 8
BS = B // N_CORES          # 1024 rows per core
P, C = 128, BS // 128      # gather tile: 128 partitions x 8 columns
CLIP = 1e-8

_cached_nc = None


def build_nc():
    global _cached_nc
    if _cached_nc is not None:
        return _cached_nc

    nc = bacc.Bacc("TRN2", target_bir_lowering=False, debug=False,
                   num_devices=N_CORES)
    probs = nc.dram_tensor("probs", [BS, V], mybir.dt.float32,
                           kind="ExternalInput")
    idx = nc.dram_tensor("idx", [P, C], mybir.dt.int32, kind="ExternalInput")
    out = nc.dram_tensor("out", [P, 1], mybir.dt.float32,
                         kind="ExternalOutput")

    probs_flat = bass.AP(probs, 0, [[1, BS * V], [1, 1]])

    with (
        nc.sbuf_tensor("idx_t", [P, C], mybir.dt.int32) as idx_t,
        nc.sbuf_tensor("g_t", [P, C], mybir.dt.float32) as g_t,
        nc.sbuf_tensor("gc_t", [P, C], mybir.dt.float32) as gc_t,
        nc.sbuf_tensor("ll_t", [P, C], mybir.dt.float32) as ll_t,
        nc.sbuf_tensor("acc_t", [P, 1], mybir.dt.float32) as acc_t,
        nc.sbuf_tensor("bias_t", [P, 1], mybir.dt.float32) as bias_t,
        nc.semaphore("s_idx") as s_idx,
        nc.semaphore("s_g") as s_g,
        nc.semaphore("s_dve") as s_dve,
        nc.semaphore("s_act") as s_act,
        nc.semaphore("s_out") as s_out,
        nc.Block() as block,
    ):
        @block.sync
        def _(sync):
            sync.dma_start(idx_t[:], idx.ap()).then_inc(s_idx, 16)
            sync.wait_ge(s_act, 1)
            sync.dma_start(out.ap(), acc_t[:]).then_inc(s_out, 16)
            sync.wait_ge(s_out, 16)

        @block.gpsimd
        def _(gpsimd):
            gpsimd.wait_ge(s_idx, 16)
            for c in range(C):
                gpsimd.indirect_dma_start(
                    out=g_t[:, c:c + 1],
                    out_offset=None,
                    in_=probs_flat,
                    in_offset=bass.IndirectOffsetOnAxis(
                        ap=idx_t[:, c:c + 1], axis=0),
                ).then_inc(s_g, 16)

        @block.vector
        def _(vector):
            vector.memset(bias_t[:], 0.0)
            vector.wait_ge(s_g, 16 * C)
            vector.tensor_scalar_max(gc_t[:], g_t[:], CLIP).then_inc(s_dve, 1)

        @block.scalar
        def _(scalar):
            scalar.wait_ge(s_dve, 1)
            scalar.activation(ll_t[:], gc_t[:],
                              mybir.ActivationFunctionType.Ln,
                              bias=bias_t[:, :1],
                              accum_out=acc_t[:]).then_inc(s_act, 1)

        sem_ids = sorted(s.num for s in (s_idx, s_g, s_dve, s_act, s_out))

    # Post-Block (after its all-engine exit barrier): reset DMA bookkeeping
    # and zero our semaphores so the NEFF can be re-executed.
    for sem_range in compact_to_ranges(sem_ids):
        nc.gpsimd.dma_reset(sem_range)
        nc.gpsimd.sem_clear(sem_range)

    nc.compile()
    _cached_nc = nc
    return nc


def make_in_maps(probs, labels):
    probs = np.ascontiguousarray(np.asarray(probs), dtype=np.float32)
    labels = np.asarray(labels).astype(np.int64, copy=False)
    assert probs.shape == (B, V) and labels.shape == (B,)
    row = np.arange(BS, dtype=np.int64) * V
    in_maps = []
    for c in range(N_CORES):
        lb = labels[c * BS:(c + 1) * BS]
        flat = (row + lb).astype(np.int32).reshape(P, C)
        in_maps.append({"probs": probs[c * BS:(c + 1) * BS], "idx": flat})
    return in_maps


def kernel(probs, labels):
    from concourse.bass_utils import run_bass_kernel_spmd
    nc = build_nc()
    in_maps = make_in_maps(probs, labels)
    res = run_bass_kernel_spmd(nc, in_maps, core_ids=list(range(N_CORES)))
    total = np.float64(0.0)
    for r in res.results:
        total += np.float64(r["out"].sum(dtype=np.float64))
    return np.array(-total / B, dtype=np.float32)


# revision 5
# speedup vs baseline: 1.0152x; 1.0152x over previous
"""No-Block variant: engine streams emitted directly into main (no exit
barrier), idx DMA split into two waves so gathers start earlier.

  SP : dma idx[:, :4] -> s_idx(16); dma idx[:, 4:] -> s_idx(32);
       wait s_act; dma out -> s_out; wait s_out
  PL : wait s_idx>=16; gathers 0-3; wait s_idx>=32; gathers 4-7 -> s_g;
       wait s_out>=16; dma_reset + sem_clear (PL saw every final value
       transitively, so no engine can still be using these sems)
  DVE: memset bias; wait s_g>=128; clamp -> s_dve
  ACT: wait s_dve; ln+accum -> s_act
"""

import numpy as np

import concourse.bacc as bacc
import concourse.bass as bass
import concourse.mybir as mybir
from concourse.bass import compact_to_ranges

B, V = 8192, 32000
N_CORES = 8
BS = B // N_CORES
P, C = 128, BS // 128
CLIP = 1e-8
H = C // 2

_cached_nc = None


def build_nc(detect_races=False):
    global _cached_nc
    if _cached_nc is not None and not detect_races:
        return _cached_nc

    nc = bacc.Bacc("TRN2", target_bir_lowering=False, debug=False,
                   num_devices=N_CORES,
                   detect_race_conditions=detect_races)
    probs = nc.dram_tensor("probs", [BS, V], mybir.dt.float32,
                           kind="ExternalInput")
    idx = nc.dram_tensor("idx", [P, C], mybir.dt.int32, kind="ExternalInput")
    out = nc.dram_tensor("out", [P, 1], mybir.dt.float32,
                         kind="ExternalOutput")

    probs_flat = bass.AP(probs, 0, [[1, BS * V], [1, 1]])

    with (
        nc.sbuf_tensor("idx_t", [P, C], mybir.dt.int32) as idx_t,
        nc.sbuf_tensor("g_t", [P, C], mybir.dt.float32) as g_t,
        nc.sbuf_tensor("gc_t", [P, C], mybir.dt.float32) as gc_t,
        nc.sbuf_tensor("ll_t", [P, C], mybir.dt.float32) as ll_t,
        nc.sbuf_tensor("acc_t", [P, 1], mybir.dt.float32) as acc_t,
        nc.sbuf_tensor("bias_t", [P, 1], mybir.dt.float32) as bias_t,
        nc.semaphore("s_idx") as s_idx,
        nc.semaphore("s_g") as s_g,
        nc.semaphore("s_dve") as s_dve,
        nc.semaphore("s_act") as s_act,
        nc.semaphore("s_out") as s_out,
    ):
        # SP stream
        nc.sync.dma_start(idx_t[:, :H], idx.ap()[:, :H]).then_inc(s_idx, 16)
        nc.sync.dma_start(idx_t[:, H:], idx.ap()[:, H:]).then_inc(s_idx, 16)
        nc.sync.wait_ge(s_act, 1)
        # No SP wait on s_out: PL's tail wait covers output landing, and a
        # second waiter could still be polling when PL clears the sem.
        nc.sync.dma_start(out.ap(), acc_t[:]).then_inc(s_out, 16)

        # PL stream
        nc.gpsimd.wait_ge(s_idx, 16)
        for c in range(H):
            nc.gpsimd.indirect_dma_start(
                out=g_t[:, c:c + 1], out_offset=None, in_=probs_flat,
                in_offset=bass.IndirectOffsetOnAxis(
                    ap=idx_t[:, c:c + 1], axis=0),
            ).then_inc(s_g, 16)
        nc.gpsimd.wait_ge(s_idx, 32)
        for c in range(H, C):
            nc.gpsimd.indirect_dma_start(
                out=g_t[:, c:c + 1], out_offset=None, in_=probs_flat,
                in_offset=bass.IndirectOffsetOnAxis(
                    ap=idx_t[:, c:c + 1], axis=0),
            ).then_inc(s_g, 16)

        # DVE stream
        nc.vector.memset(bias_t[:], 0.0)
        nc.vector.wait_ge(s_g, 16 * C)
        nc.vector.tensor_scalar_max(gc_t[:], g_t[:], CLIP).then_inc(s_dve, 1)

        # ACT stream
        nc.scalar.wait_ge(s_dve, 1)
        nc.scalar.activation(ll_t[:], gc_t[:],
                             mybir.ActivationFunctionType.Ln,
                             bias=bias_t[:, :1],
                             accum_out=acc_t[:]).then_inc(s_act, 1)

        # PL tail: by s_out>=16 every other engine's final sem value has
        # been reached and consumed, so resetting here is race-free.
        nc.gpsimd.wait_ge(s_out, 16)
        sem_ids = sorted(s.num for s in (s_idx, s_g, s_dve, s_act, s_out))
        for sem_range in compact_to_ranges(sem_ids):
            nc.gpsimd.dma_reset(sem_range)
            nc.gpsimd.sem_clear(sem_range)

    nc.compile()
    if not detect_races:
        _cached_nc = nc
    return nc


def make_in_maps(probs, labels):
    probs = np.ascontiguousarray(np.asarray(probs), dtype=np.float32)
    labels = np.asarray(labels).astype(np.int64, copy=False)
    assert probs.shape == (B, V) and labels.shape == (B,)
    row = np.arange(BS, dtype=np.int64) * V
    in_maps = []
    for c in range(N_CORES):
        lb = labels[c * BS:(c + 1) * BS]
        flat = (row + lb).astype(np.int32).reshape(P, C)
        in_maps.append({"probs": probs[c * BS:(c + 1) * BS], "idx": flat})
    return in_maps


def kernel(probs, labels):
    from concourse.bass_utils import run_bass_kernel_spmd
    nc = build_nc()
    in_maps = make_in_maps(probs, labels)
    res = run_bass_kernel_spmd(nc, in_maps, core_ids=list(range(N_CORES)))
    total = np.float64(0.0)
    for r in res.results:
        total += np.float64(r["out"].sum(dtype=np.float64))
    return np.array(-total / B, dtype=np.float32)
